# revision 34
# baseline (speedup 1.0000x reference)
"""Trainium2 Bass kernel for nn_Conv1d_NN (kNN + strided conv).

Math (per batch b):
    dist[t,s]  = ||x[:,t]||^2 + ||x[:,s]||^2 - 2 x[:,t].x[:,s]
    idx[t,:]   = top-8 smallest dist (self first), sorted ascending
    out[o,t]   = sum_{j,c} w[o,c,j] * x[c, idx[t,j]] + b[o]

Device strategy (data-parallel, 2 batches per core on 8 cores):
  - score[t,s] = 2 dot - ||x_s||^2 (row-constant shift of -dist preserves
    per-row ranking), computed at ~fp32 precision with PAIRED bf16
    matmuls: x = xh + xl (two bf16 planes, host-split), then per
    512-col chunk (one fp32 PSUM bank):
        MM1: [xh;xl]^T [2xh;2xl]   (xh.2xh + xl.2xl)
        MM2: [2xl;2xh]^T [xh;xl]   (xl.2xh + xh.2xl)
    bf16 streams at 1 cycle/row vs fp32's 4, so this is ~2x cheaper
    than one fp32 MM and ~fp32-accurate (dropped residual ~2^-18,
    ~10 wrong neighbor indices out of 262144 -> rel err ~8e-3 vs the
    2e-2 tolerance; full-bf16 scores would give 7232 wrong -> 0.22).
  - The -norm_s shift is applied off the PE to keep it under the DVE
    roofline: chunks 0,1 via a K=3 accum-matmul (3 bf16 rows
    -nhi/-nmid/-nlo), chunks 2,3 via one gpsimd tensor_add of a
    host-broadcast fp32 [128, T/2] -norm tile onto scores in SBUF.
    (A device-side gpsimd partition_broadcast triggers mid-stream
    ucode library swaps that stall the tensor_add pipeline; the DVE
    is the bottleneck engine, so everything else must stay off it.)
  - DVE max/max_index -> top-8 values + column indices per token
    (exact fp32 compare; token tiles are CONTIGUOUS 128-token slices).
    DVE cost (2 full scans/token at 1 elem/cycle/partition: MAX8 +
    FIND_INDEX8, no 16-bit speedup for the slice-serial swap chain)
    is ~146us/core and is the kernel's roofline; tensor ~115us,
    scalar ~100us, gpsimd ~78us and DMA ~50us all hide under it.
  - y[t,(j,o)] = sum_c x[c,t] w[o,c,j] via the same [xh;xl] lhsT against
    a [128,512] bf16 weight block (w rows duplicated in both planes).
    Bias is added on the host during the gather. The conv+copy are
    emitted FIRST per tile: the in-order scalar queue would otherwise
    head-of-line block the next tile's score copies behind a late conv.
  - Outputs: y (all taps, all tokens) and the top-8 index table
    ([BPC,RT,128,K] u16 - identity 2D DMA; a [T,K] layout lowers to a
    gpsimd software-DGE op that swaps ucode libraries mid-stream).
  - Inputs are prefetched for BOTH batches up front, quarter-split so
    the first tile's matmuls start early; a few dummy matmuls warm the
    PE p-state during the input DMA.

The final rank-indexed 8-way gather+sum runs on the host: this
container's runtime has no working data-dependent DMA (HIPI gpsimd
ucode excluded, DynamicAP indirect DMA generates broken descriptors),
so the O(T*K*C) permutation+sum is applied to the device-computed
y/idx tensors host-side. All matmul FLOPs (distance matrix + conv) and
the top-k run on device.
"""

import sys
import numpy as np

if "/opt/trn_rl_repo" not in sys.path:
    sys.path.insert(0, "/opt/trn_rl_repo")

B, C, T, K, OUT_C = 16, 64, 2048, 8, 64
NCORES = 8
BPC = B // NCORES  # batches per core
RT = T // 128      # 16 row tiles of 128 tokens
NF = T // 512      # 4 column chunks of 512

_CACHE = {}


def build_nc():
    import concourse.bacc as bacc
    import concourse.tile as tile
    import concourse.mybir as mybir

    dt = mybir.dt
    f32 = dt.float32
    bf16 = dt.bfloat16

    nc = bacc.Bacc(
        "TRN2", target_bir_lowering=False, debug=False, num_devices=NCORES
    )
    xp_d = nc.dram_tensor("xpair", [BPC, 128, T], bf16, kind="ExternalInput").ap()
    ra_d = nc.dram_tensor("ra", [BPC, 128, T], bf16, kind="ExternalInput").ap()
    rb_d = nc.dram_tensor("rb", [BPC, 128, T], bf16, kind="ExternalInput").ap()
    nm_d = nc.dram_tensor("nmbf", [BPC, 128, T // 2], f32, kind="ExternalInput").ap()
    nb_d = nc.dram_tensor("nrmb", [BPC, 3, T], bf16, kind="ExternalInput").ap()
    wall_d = nc.dram_tensor("wall", [128, K * OUT_C], bf16, kind="ExternalInput").ap()
    y_d = nc.dram_tensor("yout", [BPC, T, K * OUT_C], f32, kind="ExternalOutput").ap()
    gi_d = nc.dram_tensor("gidx", [BPC, RT, 128, K], dt.uint16, kind="ExternalOutput").ap()

    with tile.TileContext(nc) as tc:
        with (
            tc.tile_pool(name="const", bufs=1) as constp,
            tc.tile_pool(name="xio", bufs=2) as xio,
            tc.tile_pool(name="scoresp", bufs=4) as scp,
            tc.tile_pool(name="small", bufs=3) as smp,
            tc.tile_pool(name="yio", bufs=3) as yp,
            tc.tile_pool(name="pd", bufs=6, space="PSUM") as pdp,
            tc.tile_pool(name="py", bufs=2, space="PSUM") as pyp,
        ):
            wall_sb = constp.tile([128, K * OUT_C], bf16)
            nc.sync.dma_start(wall_sb[:], wall_d[:])
            ones3 = constp.tile([3, 128], bf16)
            nc.gpsimd.memset(ones3[:], 1.0)
            # warm the PE HAM window while input DMAs are in flight
            pwarm = pdp.tile([128, 512], f32, tag="pd", name="pwarm")
            for _ in range(3):
                nc.tensor.matmul(pwarm[:], ones3[:], wall_sb[0:3, :])

            H = T // 2
            Q = T // 4
            xps, ras, rbs, nms, nmbs = [], [], [], [], []
            for b in range(BPC):
                # split the big inputs into 512-col quarters (= chunk width)
                # so the first tile's matmuls start after a fraction of the
                # input DMA; prefetch BOTH batches before any compute.
                xpq = [
                    xio.tile([128, Q], bf16, tag=f"xpq{q}", name=f"xpq{b}_{q}")
                    for q in range(4)
                ]
                raq = [
                    xio.tile([128, Q], bf16, tag=f"raq{q}", name=f"raq{b}_{q}")
                    for q in range(4)
                ]
                rbq = [
                    xio.tile([128, Q], bf16, tag=f"rbq{q}", name=f"rbq{b}_{q}")
                    for q in range(4)
                ]
                nm = xio.tile([3, T], bf16, tag="nm", name=f"nm{b}")
                nmb = xio.tile([128, H], f32, tag="nmb", name=f"nmb{b}")
                # order: tile-0 chunk-2 deps first (lhs=xpq0, lhs2=rbq0,
                # rhs=raq2/xpq2), then nm (first tiles norm via MM3), chunk-3,
                # nmb (later tiles' gpsimd add), then the rest
                nc.sync.dma_start(xpq[0][:], xp_d[b, :, 0:Q])
                nc.sync.dma_start(rbq[0][:], rb_d[b, :, 0:Q])
                nc.sync.dma_start(raq[2][:], ra_d[b, :, 2 * Q : 3 * Q])
                nc.sync.dma_start(xpq[2][:], xp_d[b, :, 2 * Q : 3 * Q])
                nc.sync.dma_start(nm[:], nb_d[b])
                nc.sync.dma_start(raq[3][:], ra_d[b, :, 3 * Q : 4 * Q])
                nc.sync.dma_start(xpq[3][:], xp_d[b, :, 3 * Q : 4 * Q])
                nc.sync.dma_start(nmb[:], nm_d[b])
                nc.sync.dma_start(raq[0][:], ra_d[b, :, 0:Q])
                nc.sync.dma_start(xpq[1][:], xp_d[b, :, Q : 2 * Q])
                nc.sync.dma_start(raq[1][:], ra_d[b, :, Q : 2 * Q])
                for q in (1, 2, 3):
                    nc.sync.dma_start(rbq[q][:], rb_d[b, :, q * Q : (q + 1) * Q])
                xps.append(xpq)
                ras.append(raq)
                rbs.append(rbq)
                nms.append(nm)
                nmbs.append(nmb)

            # preload the gpsimd tensor_tensor ucode AFTER the broadcasts so
            # library swaps happen once at startup, never mid-stream
            scratch = constp.tile([3, 128], bf16)
            nc.gpsimd.tensor_add(scratch[:], ones3[:], ones3[:])

            for b in range(BPC):
                xpq, raq, rbq = xps[b], ras[b], rbs[b]
                nm, nmb = nms[b], nmbs[b]
                for rt in range(RT):
                    ts = slice((rt * 128) % Q, (rt * 128) % Q + 128)
                    lhs = xpq[rt // 4][:, ts]
                    lhs2 = rbq[rt // 4][:, ts]
                    # conv first: keeps the ysb copy off the critical path
                    # of the next tile's score-chunk copies (scalar queue is
                    # in-order; a late conv would head-of-line block them)
                    py = pyp.tile([128, 512], f32, tag="py", name=f"py{b}_{rt}")
                    nc.tensor.matmul(py[:], lhs, wall_sb[:])
                    ysb = yp.tile([128, 512], f32, tag="ysb", name=f"y{b}_{rt}")
                    nc.scalar.copy(ysb[:], py[:])
                    nc.scalar.dma_start(y_d[b, rt * 128 : (rt + 1) * 128, :], ysb[:])

                    scores = scp.tile([128, T], f32, tag="scores", name=f"sc{b}_{rt}")
                    # chunks 2,3 first (no PE norm matmul; gpsimd adds -norm).
                    # First two tiles: norm via MM3 on ALL chunks — the PE is
                    # idle during startup and this drops the gpsimd tensor_add
                    # from the first MAX8's critical path.
                    all_mm3 = b == 0 and rt < 2
                    for nf in (2, 3, 0, 1):
                        cs = slice(nf * 512, (nf + 1) * 512)
                        rav = raq[nf][:]
                        xpv = xpq[nf][:]
                        pd = pdp.tile([128, 512], f32, tag="pd", name=f"pd{b}_{rt}_{nf}")
                        if nf >= 2 and not all_mm3:
                            nc.tensor.matmul(pd[:], lhs, rav, start=True, stop=False)
                            nc.tensor.matmul(pd[:], lhs2, xpv, start=False, stop=True)
                        else:
                            nc.tensor.matmul(pd[:], lhs, rav, start=True, stop=False)
                            nc.tensor.matmul(pd[:], lhs2, xpv, start=False, stop=False)
                            nc.tensor.matmul(
                                pd[:], ones3[:], nm[:, cs], start=False, stop=True
                            )
                        nc.scalar.copy(scores[:, cs], pd[:])
                        if nf == 3 and not all_mm3:
                            nc.gpsimd.tensor_add(
                                scores[:, H:], scores[:, H:], nmb[:]
                            )

                    vals = smp.tile([128, 8], f32, tag="vals", name=f"v{b}_{rt}")
                    nc.vector.max(vals[:], scores[:])
                    gall = smp.tile([128, 8], dt.uint16, tag="gall", name=f"g{b}_{rt}")
                    nc.vector.max_index(gall[:], vals[:], scores[:])
                    nc.sync.dma_start(gi_d[b, rt], gall[:])

    nc.compile()
    return nc


def _get_nc():
    if "nc" not in _CACHE:
        _CACHE["nc"] = build_nc()
    return _CACHE["nc"]


def host_inputs(x, w, b):
    """Per-core input maps from full inputs."""
    import ml_dtypes

    bf = ml_dtypes.bfloat16
    x = np.asarray(x, dtype=np.float32)
    w = np.asarray(w, dtype=np.float32)
    b = np.asarray(b, dtype=np.float32)

    xh = x.astype(bf).astype(np.float32)
    xl = (x - xh).astype(bf).astype(np.float32)
    norm = (x.astype(np.float64) ** 2).sum(axis=1).astype(np.float32)  # [B, T]
    nhi = norm.astype(bf).astype(np.float32)
    nmid = (norm - nhi).astype(bf).astype(np.float32)
    nlo = (norm - nhi - nmid).astype(bf).astype(np.float32)

    xpair = np.concatenate([xh, xl], axis=1).astype(bf)           # [B, 128, T]
    ra = np.concatenate([2 * xh, 2 * xl], axis=1).astype(bf)      # [B, 128, T]
    rb = np.concatenate([2 * xl, 2 * xh], axis=1).astype(bf)      # [B, 128, T]
    # host-broadcast fp32 -norm for the second half (gpsimd tensor_add path)
    nmbf = np.ascontiguousarray(
        np.broadcast_to((-norm)[:, None, T // 2 :], (B, 128, T // 2))
    ).astype(np.float32)                                          # [B, 128, T/2]
    nrmb = np.stack([-nhi, -nmid, -nlo], axis=1).astype(bf)       # [B, 3, T]

    wr = w.transpose(1, 2, 0).reshape(C, K * OUT_C)               # [c, j*64+o]
    wall = np.concatenate([wr, wr], axis=0).astype(bf)            # [128, 512]

    return [
        {
            "xpair": np.ascontiguousarray(xpair[i * BPC : (i + 1) * BPC]),
            "ra": np.ascontiguousarray(ra[i * BPC : (i + 1) * BPC]),
            "rb": np.ascontiguousarray(rb[i * BPC : (i + 1) * BPC]),
            "nmbf": np.ascontiguousarray(nmbf[i * BPC : (i + 1) * BPC]),
            "nrmb": np.ascontiguousarray(nrmb[i * BPC : (i + 1) * BPC]),
            "wall": wall,
        }
        for i in range(NCORES)
    ]


def kernel(x, w, b):
    from concourse.bass_utils import run_bass_kernel_spmd

    nc = _get_nc()
    in_maps = host_inputs(x, w, b)

    tarange = np.arange(T, dtype=np.int64)
    for attempt in range(3):
        res = run_bass_kernel_spmd(nc, in_maps, list(range(NCORES)))
        # sanity: the self-match (distance 0) must be every token's top-1.
        # Guards against rare transient device/DMA corruption; a clean run
        # always passes this for continuous random inputs.
        ok = True
        for i in range(NCORES):
            gi = res.results[i]["gidx"]
            for bb in range(BPC):
                idx0 = gi[bb].reshape(T, K)[:, 0].astype(np.int64)
                if (idx0 != tarange).mean() > 1e-3:
                    ok = False
        if ok:
            break

    b32 = np.asarray(b, dtype=np.float32)
    out = np.empty((B, OUT_C, T), np.float32)
    jj = np.arange(K, dtype=np.int64)[None, :]
    for i in range(NCORES):
        yv = res.results[i]["yout"]    # [BPC, T, K*OUT_C]
        gi = res.results[i]["gidx"]    # [BPC, RT, 128, K] u16, t = rt*128 + p
        for bb in range(BPC):
            idx = gi[bb].reshape(T, K).astype(np.int64)   # [T, K]
            yr = yv[bb].reshape(T, K, OUT_C)              # [s, j, o]
            gathered = yr[idx, jj, :]                     # [T, K, OUT_C]
            out[i * BPC + bb] = gathered.sum(1).T + b32[:, None]
    return out.astype(np.float32)


# revision 35
# speedup vs baseline: 1.0194x; 1.0194x over previous
"""Trainium2 Bass kernel for nn_Conv1d_NN (kNN + strided conv).

Math (per batch b):
    dist[t,s]  = ||x[:,t]||^2 + ||x[:,s]||^2 - 2 x[:,t].x[:,s]
    idx[t,:]   = top-8 smallest dist (self first), sorted ascending
    out[o,t]   = sum_{j,c} w[o,c,j] * x[c, idx[t,j]] + b[o]

Device strategy (data-parallel, 2 batches per core on 8 cores):
  - score[t,s] = 2 dot - ||x_s||^2 (row-constant shift of -dist preserves
    per-row ranking), computed at ~fp32 precision with PAIRED bf16
    matmuls: x = xh + xl (two bf16 planes, host-split), then per
    512-col chunk (one fp32 PSUM bank):
        MM1: [xh;xl]^T [2xh;2xl]   (xh.2xh + xl.2xl)
        MM2: [2xl;2xh]^T [xh;xl]   (xl.2xh + xh.2xl)
    bf16 streams at 1 cycle/row vs fp32's 4, so this is ~2x cheaper
    than one fp32 MM and ~fp32-accurate (dropped residual ~2^-18,
    ~10 wrong neighbor indices out of 262144 -> rel err ~8e-3 vs the
    2e-2 tolerance; full-bf16 scores would give 7232 wrong -> 0.22).
  - The -norm_s shift is applied off the PE to keep it under the DVE
    roofline: chunks 0,1 via a K=3 accum-matmul (3 bf16 rows
    -nhi/-nmid/-nlo), chunks 2,3 via one gpsimd tensor_add of a
    host-broadcast fp32 [128, T/2] -norm tile onto scores in SBUF.
    (A device-side gpsimd partition_broadcast triggers mid-stream
    ucode library swaps that stall the tensor_add pipeline; the DVE
    is the bottleneck engine, so everything else must stay off it.)
  - DVE max/max_index -> top-8 values + column indices per token
    (exact fp32 compare; token tiles are CONTIGUOUS 128-token slices).
    DVE cost (2 full scans/token at 1 elem/cycle/partition: MAX8 +
    FIND_INDEX8, no 16-bit speedup for the slice-serial swap chain)
    is ~146us/core and is the kernel's roofline; tensor ~115us,
    scalar ~100us, gpsimd ~78us and DMA ~50us all hide under it.
  - y[t,(j,o)] = sum_c x[c,t] w[o,c,j] via the same [xh;xl] lhsT against
    a [128,512] bf16 weight block (w rows duplicated in both planes).
    Bias is added on the host during the gather. The conv+copy are
    emitted FIRST per tile: the in-order scalar queue would otherwise
    head-of-line block the next tile's score copies behind a late conv.
  - Outputs: y (all taps, all tokens) and the top-8 index table
    ([BPC,RT,128,K] u16 - identity 2D DMA; a [T,K] layout lowers to a
    gpsimd software-DGE op that swaps ucode libraries mid-stream).
  - Inputs are prefetched for BOTH batches up front, quarter-split so
    the first tile's matmuls start early; a few dummy matmuls warm the
    PE p-state during the input DMA.

The final rank-indexed 8-way gather+sum runs on the host: this
container's runtime has no working data-dependent DMA (HIPI gpsimd
ucode excluded, DynamicAP indirect DMA generates broken descriptors),
so the O(T*K*C) permutation+sum is applied to the device-computed
y/idx tensors host-side. All matmul FLOPs (distance matrix + conv) and
the top-k run on device.
"""

import sys
import numpy as np

if "/opt/trn_rl_repo" not in sys.path:
    sys.path.insert(0, "/opt/trn_rl_repo")

B, C, T, K, OUT_C = 16, 64, 2048, 8, 64
NCORES = 8
BPC = B // NCORES  # batches per core
RT = T // 128      # 16 row tiles of 128 tokens
NF = T // 512      # 4 column chunks of 512

_CACHE = {}


def build_nc():
    import concourse.bacc as bacc
    import concourse.tile as tile
    import concourse.mybir as mybir

    dt = mybir.dt
    f32 = dt.float32
    bf16 = dt.bfloat16

    nc = bacc.Bacc(
        "TRN2", target_bir_lowering=False, debug=False, num_devices=NCORES
    )
    xp_d = nc.dram_tensor("xpair", [BPC, 128, T], bf16, kind="ExternalInput").ap()
    ra_d = nc.dram_tensor("ra", [BPC, 128, T], bf16, kind="ExternalInput").ap()
    rb_d = nc.dram_tensor("rb", [BPC, 128, T], bf16, kind="ExternalInput").ap()
    nm_d = nc.dram_tensor("nmbf", [BPC, 128, T // 2], f32, kind="ExternalInput").ap()
    nb_d = nc.dram_tensor("nrmb", [BPC, 3, T], bf16, kind="ExternalInput").ap()
    wall_d = nc.dram_tensor("wall", [128, K * OUT_C], bf16, kind="ExternalInput").ap()
    y_d = nc.dram_tensor("yout", [BPC, T, K * OUT_C], f32, kind="ExternalOutput").ap()
    gi_d = nc.dram_tensor("gidx", [BPC, RT, 128, K], dt.uint16, kind="ExternalOutput").ap()

    with tile.TileContext(nc) as tc:
        with (
            tc.tile_pool(name="const", bufs=1) as constp,
            tc.tile_pool(name="xio", bufs=2) as xio,
            tc.tile_pool(name="scoresp", bufs=4) as scp,
            tc.tile_pool(name="small", bufs=3) as smp,
            tc.tile_pool(name="yio", bufs=3) as yp,
            tc.tile_pool(name="pd", bufs=6, space="PSUM") as pdp,
            tc.tile_pool(name="py", bufs=2, space="PSUM") as pyp,
        ):
            wall_sb = constp.tile([128, K * OUT_C], bf16)
            nc.sync.dma_start(wall_sb[:], wall_d[:])
            ones3 = constp.tile([3, 128], bf16)
            nc.gpsimd.memset(ones3[:], 1.0)
            # warm the PE HAM window while input DMAs are in flight
            pwarm = pdp.tile([128, 512], f32, tag="pd", name="pwarm")
            for _ in range(3):
                nc.tensor.matmul(pwarm[:], ones3[:], wall_sb[0:3, :])

            H = T // 2
            Q = T // 4
            xps, ras, rbs, nms, nmbs = [], [], [], [], []
            for b in range(BPC):
                # split the big inputs into 512-col quarters (= chunk width)
                # so the first tile's matmuls start after a fraction of the
                # input DMA; prefetch BOTH batches before any compute.
                xpq = [
                    xio.tile([128, Q], bf16, tag=f"xpq{q}", name=f"xpq{b}_{q}")
                    for q in range(4)
                ]
                raq = [
                    xio.tile([128, Q], bf16, tag=f"raq{q}", name=f"raq{b}_{q}")
                    for q in range(4)
                ]
                rbq = [
                    xio.tile([128, Q], bf16, tag=f"rbq{q}", name=f"rbq{b}_{q}")
                    for q in range(4)
                ]
                nm = xio.tile([3, T], bf16, tag="nm", name=f"nm{b}")
                nmb = xio.tile([128, H], f32, tag="nmb", name=f"nmb{b}")
                # order: tile-0 chunk-2 deps first (lhs=xpq0, lhs2=rbq0,
                # rhs=raq2/xpq2), then nm (first tiles norm via MM3), chunk-3,
                # nmb (later tiles' gpsimd add), then the rest
                nc.sync.dma_start(xpq[0][:], xp_d[b, :, 0:Q])
                nc.sync.dma_start(rbq[0][:], rb_d[b, :, 0:Q])
                nc.sync.dma_start(raq[2][:], ra_d[b, :, 2 * Q : 3 * Q])
                nc.sync.dma_start(xpq[2][:], xp_d[b, :, 2 * Q : 3 * Q])
                nc.sync.dma_start(nm[:], nb_d[b])
                nc.sync.dma_start(raq[3][:], ra_d[b, :, 3 * Q : 4 * Q])
                nc.sync.dma_start(xpq[3][:], xp_d[b, :, 3 * Q : 4 * Q])
                nc.sync.dma_start(nmb[:], nm_d[b])
                nc.sync.dma_start(raq[0][:], ra_d[b, :, 0:Q])
                nc.sync.dma_start(xpq[1][:], xp_d[b, :, Q : 2 * Q])
                nc.sync.dma_start(raq[1][:], ra_d[b, :, Q : 2 * Q])
                for q in (1, 2, 3):
                    nc.sync.dma_start(rbq[q][:], rb_d[b, :, q * Q : (q + 1) * Q])
                xps.append(xpq)
                ras.append(raq)
                rbs.append(rbq)
                nms.append(nm)
                nmbs.append(nmb)

            # preload the gpsimd tensor_tensor ucode AFTER the broadcasts so
            # library swaps happen once at startup, never mid-stream
            scratch = constp.tile([3, 128], bf16)
            nc.gpsimd.tensor_add(scratch[:], ones3[:], ones3[:])

            for b in range(BPC):
                xpq, raq, rbq = xps[b], ras[b], rbs[b]
                nm, nmb = nms[b], nmbs[b]
                for rt in range(RT):
                    ts = slice((rt * 128) % Q, (rt * 128) % Q + 128)
                    lhs = xpq[rt // 4][:, ts]
                    lhs2 = rbq[rt // 4][:, ts]
                    # conv first: keeps the ysb copy off the critical path
                    # of the next tile's score-chunk copies (scalar queue is
                    # in-order; a late conv would head-of-line block them)
                    py = pyp.tile([128, 512], f32, tag="py", name=f"py{b}_{rt}")
                    nc.tensor.matmul(py[:], lhs, wall_sb[:])
                    ysb = yp.tile([128, 512], f32, tag="ysb", name=f"y{b}_{rt}")
                    nc.scalar.copy(ysb[:], py[:])
                    nc.scalar.dma_start(y_d[b, rt * 128 : (rt + 1) * 128, :], ysb[:])

                    scores = scp.tile([128, T], f32, tag="scores", name=f"sc{b}_{rt}")
                    # chunks 2,3 first (no PE norm matmul; gpsimd adds -norm)
                    all_mm3 = False
                    for nf in (2, 3, 0, 1):
                        cs = slice(nf * 512, (nf + 1) * 512)
                        rav = raq[nf][:]
                        xpv = xpq[nf][:]
                        pd = pdp.tile([128, 512], f32, tag="pd", name=f"pd{b}_{rt}_{nf}")
                        if nf >= 2 and not all_mm3:
                            nc.tensor.matmul(pd[:], lhs, rav, start=True, stop=False)
                            nc.tensor.matmul(pd[:], lhs2, xpv, start=False, stop=True)
                        else:
                            nc.tensor.matmul(pd[:], lhs, rav, start=True, stop=False)
                            nc.tensor.matmul(pd[:], lhs2, xpv, start=False, stop=False)
                            nc.tensor.matmul(
                                pd[:], ones3[:], nm[:, cs], start=False, stop=True
                            )
                        nc.scalar.copy(scores[:, cs], pd[:])
                        if nf == 3 and not all_mm3:
                            nc.gpsimd.tensor_add(
                                scores[:, H:], scores[:, H:], nmb[:]
                            )

                    vals = smp.tile([128, 8], f32, tag="vals", name=f"v{b}_{rt}")
                    nc.vector.max(vals[:], scores[:])
                    gall = smp.tile([128, 8], dt.uint16, tag="gall", name=f"g{b}_{rt}")
                    nc.vector.max_index(gall[:], vals[:], scores[:])
                    nc.sync.dma_start(gi_d[b, rt], gall[:])

    nc.compile()
    return nc


def _get_nc():
    if "nc" not in _CACHE:
        _CACHE["nc"] = build_nc()
    return _CACHE["nc"]


def host_inputs(x, w, b):
    """Per-core input maps from full inputs."""
    import ml_dtypes

    bf = ml_dtypes.bfloat16
    x = np.asarray(x, dtype=np.float32)
    w = np.asarray(w, dtype=np.float32)
    b = np.asarray(b, dtype=np.float32)

    xh = x.astype(bf).astype(np.float32)
    xl = (x - xh).astype(bf).astype(np.float32)
    norm = (x.astype(np.float64) ** 2).sum(axis=1).astype(np.float32)  # [B, T]
    nhi = norm.astype(bf).astype(np.float32)
    nmid = (norm - nhi).astype(bf).astype(np.float32)
    nlo = (norm - nhi - nmid).astype(bf).astype(np.float32)

    xpair = np.concatenate([xh, xl], axis=1).astype(bf)           # [B, 128, T]
    ra = np.concatenate([2 * xh, 2 * xl], axis=1).astype(bf)      # [B, 128, T]
    rb = np.concatenate([2 * xl, 2 * xh], axis=1).astype(bf)      # [B, 128, T]
    # host-broadcast fp32 -norm for the second half (gpsimd tensor_add path)
    nmbf = np.ascontiguousarray(
        np.broadcast_to((-norm)[:, None, T // 2 :], (B, 128, T // 2))
    ).astype(np.float32)                                          # [B, 128, T/2]
    nrmb = np.stack([-nhi, -nmid, -nlo], axis=1).astype(bf)       # [B, 3, T]

    wr = w.transpose(1, 2, 0).reshape(C, K * OUT_C)               # [c, j*64+o]
    wall = np.concatenate([wr, wr], axis=0).astype(bf)            # [128, 512]

    return [
        {
            "xpair": np.ascontiguousarray(xpair[i * BPC : (i + 1) * BPC]),
            "ra": np.ascontiguousarray(ra[i * BPC : (i + 1) * BPC]),
            "rb": np.ascontiguousarray(rb[i * BPC : (i + 1) * BPC]),
            "nmbf": np.ascontiguousarray(nmbf[i * BPC : (i + 1) * BPC]),
            "nrmb": np.ascontiguousarray(nrmb[i * BPC : (i + 1) * BPC]),
            "wall": wall,
        }
        for i in range(NCORES)
    ]


def kernel(x, w, b):
    from concourse.bass_utils import run_bass_kernel_spmd

    nc = _get_nc()
    in_maps = host_inputs(x, w, b)

    tarange = np.arange(T, dtype=np.int64)
    for attempt in range(3):
        res = run_bass_kernel_spmd(nc, in_maps, list(range(NCORES)))
        # sanity: the self-match (distance 0) must be every token's top-1.
        # Guards against rare transient device/DMA corruption; a clean run
        # always passes this for continuous random inputs.
        ok = True
        for i in range(NCORES):
            gi = res.results[i]["gidx"]
            for bb in range(BPC):
                idx0 = gi[bb].reshape(T, K)[:, 0].astype(np.int64)
                if (idx0 != tarange).mean() > 1e-3:
                    ok = False
        if ok:
            break

    b32 = np.asarray(b, dtype=np.float32)
    out = np.empty((B, OUT_C, T), np.float32)
    jj = np.arange(K, dtype=np.int64)[None, :]
    for i in range(NCORES):
        yv = res.results[i]["yout"]    # [BPC, T, K*OUT_C]
        gi = res.results[i]["gidx"]    # [BPC, RT, 128, K] u16, t = rt*128 + p
        for bb in range(BPC):
            idx = gi[bb].reshape(T, K).astype(np.int64)   # [T, K]
            yr = yv[bb].reshape(T, K, OUT_C)              # [s, j, o]
            gathered = yr[idx, jj, :]                     # [T, K, OUT_C]
            out[i * BPC + bb] = gathered.sum(1).T + b32[:, None]
    return out.astype(np.float32)
